# revision 17
# baseline (speedup 1.0000x reference)
"""Multi-head attention kernel for 8 TRN2 NeuronCores.

Problem: x[4,2048,1024] -> qkv proj (w_qkv[1024,3072]) -> 16-head attention
(dim_head=64, scale=1024**-0.5) -> out proj (w_out[1024,1024] + b_out).

Sharding: core c in 0..7 handles batch b=c//2, head-group g=c%2 (8 heads).
Each core computes a partial output y_partial = attn_out_g @ w_out[rows_g];
host sums the pair (the tensor-parallel all-reduce, done at unshard time).

Layout strategy (zero on-chip transposes):
  - host supplies xT = x[b].T                     [1024, 2048] fp16
  - qkT = (x @ w_qk).T computed via chains:  lhsT=w chunk, rhs=xT -> [c, i]
    cast to fp8e4m3, then DMA-relaid to a DoubleRow layout
    qkdr[c][64, 2, N]: partitions 0:32 = even head (dh split in free dim 1),
    32:64 = odd head.
  - V   = x @ w_v computed normally:       lhsT=xT chunk, rhs=wv -> [i, c]
  - S^T = k_h @ q_h^T per head as ONE fp8 DoubleRow matmul (K=2x32):
          lhsT=kdr slice [32,2,128], rhs=qdr slice [32,2,512] -> [keys, q]
  - P   = exp(S^T * scale); most tiles on ScalarE (no max subtraction:
    |S*scale| < ~0.8); a few tiles per pass on the DVE via the quadratic
    0.5*(S*scale+1)^2 + 0.5 (|S*scale| sigma ~0.12 -> rel err ~1e-3),
    relieving the saturated ScalarE exp stream.
  - O^T|s = [v_h | 1]^T @ P : lhsT=v[128,65] (ones col), rhs=P -> [65, q]
          row 64 is the softmax denominator s
  - 1/s via reciprocal_approx_fast straight off PSUM (fp32), bounced
    through DRAM and broadcast-DMA'd across partitions
  - y = sum_h (O_h^T).T @ w_out_h : lhsT=otn[64,128], rhs=wo -> [i, e];
    proj chains for finished query blocks interleave under the last
    head-pair's passes so the tail is short.
All PE inputs fp16 except S^T (fp8e4m3 DoubleRow); PSUM fp32, output fp32.
"""

import numpy as np

B, N, D = 4, 2048, 1024
HEADS, DH = 16, 64
HP = HEADS // 2          # heads per core
GDIM = HP * DH           # 512 columns per head-group
SCALE = float(D) ** -0.5
NCORES = 8

USE_DR = False            # fp8e4m3 DoubleRow S^T matmuls
DVE_KCS = ()         # kc indices whose exp runs on the DVE (quadratic)

_CACHE = {}


def _build():
    from contextlib import ExitStack

    import concourse.bass as bass
    import concourse.tile as tile
    from concourse import bacc, mybir

    F8 = mybir.dt.float8e4
    F16 = mybir.dt.float16
    F32 = mybir.dt.float32
    EXP = mybir.ActivationFunctionType.Exp
    MULT = mybir.AluOpType.mult
    ADD = mybir.AluOpType.add
    DR = mybir.MatmulPerfMode.DoubleRow

    nc = bacc.Bacc(None, target_bir_lowering=False)

    xT_d = nc.declare_dram_parameter("xT", [D, N], F16, isOutput=False)
    wqk_d = nc.declare_dram_parameter("wqk", [D, 2 * GDIM], F16, isOutput=False)
    wv_d = nc.declare_dram_parameter("wv", [D, GDIM], F16, isOutput=False)
    wo_d = nc.declare_dram_parameter("wo", [4, 128, D], F16, isOutput=False)
    bias_d = nc.declare_dram_parameter("bias", [D], F32, isOutput=False)
    out_d = nc.declare_dram_parameter("out", [N, D], F32, isOutput=True)

    with tile.TileContext(nc) as tc, ExitStack() as ctx:
        persist = ctx.enter_context(tc.tile_pool(name="persist", bufs=1))
        ptp = ctx.enter_context(tc.tile_pool(name="ptp", bufs=8))
        quadp = ctx.enter_context(tc.tile_pool(name="quadp", bufs=2))
        rawp = ctx.enter_context(tc.tile_pool(name="rawp", bufs=5))
        tiny = ctx.enter_context(tc.tile_pool(name="tiny", bufs=4))
        ypool = ctx.enter_context(tc.tile_pool(name="ypool", bufs=2))
        dramp = ctx.enter_context(tc.tile_pool(name="dramp", bufs=4,
                                               space="DRAM"))
        # PSUM budget (8 banks): stq [128,1024] x2 bufs = 4 banks,
        # ot0/ot1/qf0/qf1 1 bank each.
        mm = ctx.enter_context(tc.tile_pool(name="mm", bufs=2, space="PSUM"))
        acc = ctx.enter_context(tc.tile_pool(name="acc", bufs=1, space="PSUM"))

        # ---- persistent SBUF tiles -------------------------------------
        xT = [persist.tile([128, N], F16, name=f"xT{e}", tag=f"xT{e}")
              for e in range(8)]
        wqk = [persist.tile([128, 2 * GDIM], F16, name=f"wqk{e}", tag=f"wqk{e}")
               for e in range(8)]
        wv = [persist.tile([128, GDIM], F16, name=f"wv{e}", tag=f"wv{e}")
              for e in range(8)]
        wo = [persist.tile([128, D], F16, name=f"wo{tp}", tag=f"wo{tp}")
              for tp in range(4)]
        bias = persist.tile([128, D], F32, tag="bias")
        QDT = F8 if USE_DR else F16
        qkT = [persist.tile([128, N], QDT, name=f"qkT{c}", tag=f"qkT{c}")
               for c in range(8)]
        if USE_DR:
            qkdr = [persist.tile([64, 2, N], F8, name=f"qkdr{c}",
                                 tag=f"qkdr{c}") for c in range(8)]
        vt = [persist.tile([128, HP, DH + 1], F16, name=f"v{kc}", tag=f"v{kc}")
              for kc in range(16)]
        otn = [persist.tile([128, N], F16, name=f"otn{tp}", tag=f"otn{tp}")
               for tp in range(4)]

        inq = (nc.sync, nc.scalar, nc.gpsimd)
        for e in range(8):
            inq[e % 3].dma_start(out=xT[e], in_=xT_d[e * 128:(e + 1) * 128, :])
            inq[(e + 1) % 3].dma_start(out=wv[e],
                                       in_=wv_d[e * 128:(e + 1) * 128, :])
        for e in range(8):
            inq[(e + 2) % 3].dma_start(out=wqk[e],
                                       in_=wqk_d[e * 128:(e + 1) * 128, :])
        for tp in range(4):
            inq[tp % 3].dma_start(out=wo[tp], in_=wo_d[tp])
        bias_ap = bias_d[:]
        nc.sync.dma_start(
            out=bias,
            in_=bass.AP(tensor=bias_ap.tensor, offset=bias_ap.offset,
                        ap=[[0, 128]] + list(bias_ap.ap)),
        )
        for kc in range(16):
            nc.vector.memset(vt[kc][:, :, 0:1], 1.0)

        # ---- PE warm-up: dummy matmuls during the input-DMA window ------
        wu = persist.tile([128, 512], F16, tag="wu")
        nc.vector.memset(wu, 0.0)
        wps = mm.tile([128, 1024], F32, name="stq", tag="stq")
        for r in range(24):
            nc.tensor.matmul(wps[:, 0:256], lhsT=wu[:, 0:128],
                             rhs=wu[:, 0:256], start=True, stop=True)

        # ---- chains: V and qkT; qkT quarters relaid to DR layout -------
        PSLOTS = ["ot0", "ot1", "qf0", "qf1"]
        rl_q = [nc.scalar, nc.sync]
        rl_i = [0]

        def emit_relayout(c, iq):
            """DMA qkT fp8 quarter iq into the DoubleRow layout."""
            sl = slice(iq * 512, (iq + 1) * 512)
            for head in (0, 1):
                for h in (0, 1):
                    src = qkT[c][64 * head + 32 * h:64 * head + 32 * h + 32,
                                 sl]
                    dst = qkdr[c][32 * head:32 * head + 32, h, sl]
                    q = rl_q[rl_i[0] % 2]
                    rl_i[0] += 1
                    q.dma_start(out=dst, in_=src)

        def v_chain_small(it, slot):
            ps = acc.tile([128, 512], F32, name=f"pv{it}",
                          tag=PSLOTS[slot % 4])
            for e in range(8):
                yield nc.tensor.matmul(
                    ps, lhsT=xT[e][:, it * 128:(it + 1) * 128],
                    rhs=wv[e], start=(e == 0), stop=(e == 7))
            yield nc.vector.tensor_copy(
                vt[it][:, :, 1:DH + 1],
                ps.rearrange("p (h d) -> p h d", h=HP))

        def qkv_chain_small(c, iq, slot):
            ps = acc.tile([128, 512], F32, name=f"pq{c}_{iq}",
                          tag=PSLOTS[slot % 4])
            for e in range(8):
                yield nc.tensor.matmul(
                    ps, lhsT=wqk[e][:, c * 128:(c + 1) * 128],
                    rhs=xT[e][:, iq * 512:(iq + 1) * 512],
                    start=(e == 0), stop=(e == 7))
            yield nc.vector.tensor_copy(
                qkT[c][:, iq * 512:(iq + 1) * 512], ps)
            if USE_DR:
                emit_relayout(c, iq)

        gens = []
        for it in range(16):
            gens.append(("v", it))
        for c in (0, 4):
            for iq in range(4):
                gens.append(("qk", c, iq))
        streams = []
        slot_rr = 0
        for g in gens:
            if g[0] == "v":
                streams.append(v_chain_small(g[1], slot_rr % 4))
            else:
                streams.append(qkv_chain_small(g[1], g[2], slot_rr % 4))
            slot_rr += 1
        live = streams[:6]
        nxt = 6
        while live:
            done = []
            for s in live:
                if next(s, None) is None:
                    done.append(s)
            for s in done:
                live.remove(s)
                if nxt < len(streams):
                    live.append(streams[nxt])
                    nxt += 1

        # ---- attention: head pairs x q-quarters. Each stq tile holds both
        # heads' scores side by side ([A 512 | B 512]) so one FD=1024 exp
        # covers the pair. fp8 DoubleRow makes each head's S^T one matmul. --
        def emit_st_exp(t, qc, kc, use_dve=False):
            qch, kch = t, 4 + t
            stq = mm.tile([128, 1024], F32, name="stq", tag="stq")
            if USE_DR:
                nc.tensor.matmul(
                    stq[:, 0:512],
                    lhsT=qkdr[kch][0:32, :, kc * 128:(kc + 1) * 128],
                    rhs=qkdr[qch][0:32, :, qc * 512:(qc + 1) * 512],
                    start=True, stop=True, perf_mode=DR)
                nc.tensor.matmul(
                    stq[:, 512:1024],
                    lhsT=qkdr[kch][32:64, :, kc * 128:(kc + 1) * 128],
                    rhs=qkdr[qch][32:64, :, qc * 512:(qc + 1) * 512],
                    start=True, stop=True, perf_mode=DR)
            else:
                nc.tensor.matmul(
                    stq[:, 0:512],
                    lhsT=qkT[kch][0:64, kc * 128:(kc + 1) * 128],
                    rhs=qkT[qch][0:64, qc * 512:(qc + 1) * 512],
                    start=True, stop=True)
                nc.tensor.matmul(
                    stq[:, 512:1024],
                    lhsT=qkT[kch][64:128, kc * 128:(kc + 1) * 128],
                    rhs=qkT[qch][64:128, qc * 512:(qc + 1) * 512],
                    start=True, stop=True)
            pt = ptp.tile([128, 1024], F16, name="pt", tag="pt")
            if use_dve:
                # exp(x) ~= 0.5*(x+1)^2 + 0.5 for x = S*SCALE (|x| small)
                w_ = quadp.tile([128, 1024], F16, name="qw", tag="qw")
                nc.vector.tensor_scalar(out=w_, in0=stq, scalar1=SCALE,
                                        scalar2=1.0, op0=MULT, op1=ADD)
                v_ = quadp.tile([128, 1024], F16, name="qv", tag="qv")
                nc.vector.tensor_mul(v_, w_, w_)
                nc.vector.tensor_scalar(out=pt, in0=v_, scalar1=0.5,
                                        scalar2=0.5, op0=MULT, op1=ADD)
            else:
                nc.scalar.activation(pt, stq, EXP, scale=SCALE)
            return pt

        # filler state: remaining qkT chunks as chains (8 MM + copy)
        fill_specs = []
        for tt in range(1, 4):
            for c in (tt, 4 + tt):
                for iq in range(4):
                    fill_specs.append((c, iq))

        def fill_chain(c, iq, slot):
            ps = acc.tile([128, 512], F32, name=f"qf{c}_{iq}",
                          tag=f"qf{slot}")
            for e in range(8):
                yield nc.tensor.matmul(
                    ps, lhsT=wqk[e][:, c * 128:(c + 1) * 128],
                    rhs=xT[e][:, iq * 512:(iq + 1) * 512],
                    start=(e == 0), stop=(e == 7))
            yield nc.vector.tensor_copy(
                qkT[c][:, iq * 512:(iq + 1) * 512], ps)
            if USE_DR:
                emit_relayout(c, iq)

        fill_state = {"gen": None, "idx": 0, "slot": 0}

        def emit_fill(n):
            for _ in range(n):
                while True:
                    if fill_state["gen"] is None:
                        if fill_state["idx"] >= len(fill_specs):
                            return False
                        c, iq = fill_specs[fill_state["idx"]]
                        fill_state["idx"] += 1
                        fill_state["slot"] ^= 1
                        if c == "v":
                            fill_state["gen"] = v_chain_small(
                                iq, 2 + fill_state["slot"])
                        else:
                            fill_state["gen"] = fill_chain(c, iq,
                                                           fill_state["slot"])
                    if next(fill_state["gen"], None) is None:
                        fill_state["gen"] = None
                        continue
                    break
            return True

        # ---- output projection chains (interleaved under late passes) ---
        yq = [nc.sync, nc.sync]

        def proj_chain(it, half, tag):
            ps = acc.tile([128, 512], F32, name=f"pj{it}_{half}",
                          tag=tag)
            e0 = half * 512
            for tp in range(4):
                yield nc.tensor.matmul(
                    ps, lhsT=otn[tp][:, it * 128:(it + 1) * 128],
                    rhs=wo[tp][:, e0:e0 + 512],
                    start=(tp == 0), stop=(tp == 3))
            yt = ypool.tile([128, 512], F32, name="yt", tag="yt", bufs=4)
            yield nc.vector.tensor_add(yt, ps, bias[:, e0:e0 + 512])
            yield yq[(it + half) % 2].dma_start(
                out=out_d[it * 128:(it + 1) * 128, e0:e0 + 512], in_=yt)

        proj_state = {"gen": None, "specs": [], "idx": 0, "slot": 0,
                      "tags": ["qf0", "qf1"]}

        def emit_proj(n):
            for _ in range(n):
                while True:
                    if proj_state["gen"] is None:
                        if proj_state["idx"] >= len(proj_state["specs"]):
                            return False
                        it, half = proj_state["specs"][proj_state["idx"]]
                        proj_state["idx"] += 1
                        proj_state["slot"] += 1
                        tags = proj_state["tags"]
                        proj_state["gen"] = proj_chain(
                            it, half, tags[proj_state["slot"] % len(tags)])
                    if next(proj_state["gen"], None) is None:
                        proj_state["gen"] = None
                        continue
                    break
            return True

        def emit_work(n):
            """Fill from qkT chains first, then proj chains."""
            for _ in range(n):
                if emit_fill(1):
                    continue
                if not emit_proj(1):
                    return

        passes = [(t, qc) for t in range(4) for qc in range(4)]
        hoisted = []
        for pi, (t, qc) in enumerate(passes):
            hA, hB = 2 * t, 2 * t + 1
            otA = acc.tile([65, 512], F32, name=f"otA{pi}", tag="ot0")
            otB = acc.tile([65, 512], F32, name=f"otB{pi}", tag="ot1")

            def emit_ot(kc, pt):
                st, sp = (kc == 0), (kc == 15)
                nc.tensor.matmul(otA, lhsT=vt[kc][:, hA, :],
                                 rhs=pt[:, 0:512], start=st, stop=sp,
                                 skip_group_check=True)
                nc.tensor.matmul(otB, lhsT=vt[kc][:, hB, :],
                                 rhs=pt[:, 512:1024], start=st, stop=sp,
                                 skip_group_check=True)

            pt_hist = []
            kc_start = len(hoisted)
            for hk, hp in hoisted:
                pt_hist.append((hk, hp))
            hoisted = []
            for kc in range(kc_start, 16):
                if pi == 0 and kc in (0, 1, 2):
                    for dj in (0, 1):
                        nc.tensor.matmul([otA, otB][dj], lhsT=wu[:, 0:65],
                                         rhs=wu, start=True, stop=True,
                                         skip_group_check=True)
                pt = emit_st_exp(t, qc, kc,
                                 use_dve=(pi >= 7 and kc in DVE_KCS))
                pt_hist.append((kc, pt))
                if len(pt_hist) > 2:
                    k2, p2 = pt_hist.pop(0)
                    emit_ot(k2, p2)
                emit_work(2 if t == 3 else 1)
            if pi + 1 < len(passes):
                nt_, nqc = passes[pi + 1]
                for hk in (0, 1):
                    hoisted.append(
                        (hk, emit_st_exp(nt_, nqc, hk)))
            for k2, p2 in pt_hist:
                emit_ot(k2, p2)
            emit_work(8 if t == 3 else 4)

            # normalize the two heads (off critical path). The ones
            # column sits FIRST in vt, so the denominator s is PSUM row 0
            # (reciprocal_approx_fast mis-executes on high base partitions)
            # and O^T occupies rows 1:65, which both heads bounce through
            # DRAM into a single [128,512] tile for one fused multiply.
            raws, rcs = {}, {}
            for j, ott in enumerate((otA, otB)):
                raw = rawp.tile([65, 512], F16, name="raw", tag="raw")
                nc.vector.tensor_copy(raw, ott)
                raws[j] = raw
            for j, ott in enumerate((otA, otB)):
                rc = tiny.tile([1, 512], F32, name="rc", tag="rc", bufs=4)
                nc.vector.reciprocal_approx_fast(rc[0:1, :], ott[0:1, :])
                rcs[j] = rc
            bc = tiny.tile([128, 512], F32, name="bc", tag="bc")
            sh = rawp.tile([128, 512], F16, name="sh", tag="sh", bufs=2)
            for j in (0, 1):
                po = 64 * j
                dsc = dramp.tile([512], F32, name="dsc", tag="dsc")
                nc.sync.dma_start(out=dsc, in_=rcs[j][0:1, :])
                dap = dsc[:]
                nc.sync.dma_start(
                    out=bc[po:po + 64, :],
                    in_=bass.AP(tensor=dap.tensor, offset=dap.offset,
                                ap=[[0, 64]] + list(dap.ap)))
                rdsc = dramp.tile([64, 512], F16, name="rdsc",
                                  tag="rdsc", bufs=2)
                nc.sync.dma_start(out=rdsc, in_=raws[j][1:65, :])
                nc.sync.dma_start(out=sh[po:po + 64, :], in_=rdsc[:])
            nc.gpsimd.tensor_mul(
                otn[t][:, qc * 512:(qc + 1) * 512], sh, bc)

            # queue the proj chains for this query block once the last
            # head-pair's normalize for it has been emitted
            if t == 3:
                for it in range(4 * qc, 4 * qc + 4):
                    proj_state["specs"].append((it, 0))
                    proj_state["specs"].append((it, 1))

        # drain whatever work remains (last query block's projections);
        # the OT banks are free now, so give the drain all four slots
        while emit_fill(1):
            pass
        proj_state["tags"] = ["qf0", "qf1", "ot0", "ot1"]
        while emit_proj(1):
            pass

    nc.compile()
    return nc


def _in_maps(x, w_qkv, w_out, b_out):
    x = np.asarray(x, dtype=np.float32)
    w_qkv = np.asarray(w_qkv, dtype=np.float32)
    w_out = np.asarray(w_out, dtype=np.float32)
    b_out = np.asarray(b_out, dtype=np.float32)
    maps = []
    for c in range(NCORES):
        b, g = c // 2, c % 2
        qcols = w_qkv[:, g * GDIM:(g + 1) * GDIM]
        kcols = w_qkv[:, D + g * GDIM:D + (g + 1) * GDIM]
        vcols = w_qkv[:, 2 * D + g * GDIM:2 * D + (g + 1) * GDIM]
        maps.append({
            "xT": np.ascontiguousarray(x[b].T).astype(np.float16),
            "wqk": np.concatenate([qcols, kcols], axis=1).astype(np.float16),
            "wv": np.ascontiguousarray(vcols).astype(np.float16),
            "wo": np.ascontiguousarray(
                w_out[g * GDIM:(g + 1) * GDIM, :].reshape(4, 128, D)
            ).astype(np.float16),
            "bias": (b_out if g == 0 else np.zeros_like(b_out)),
        })
    return maps


def kernel(x, w_qkv, w_out, b_out):
    from concourse.bass_utils import run_bass_kernel_spmd

    if "nc" not in _CACHE:
        _CACHE["nc"] = _build()
    nc = _CACHE["nc"]
    maps = _in_maps(x, w_qkv, w_out, b_out)
    res = run_bass_kernel_spmd(nc, maps, core_ids=list(range(NCORES)))
    outs = res.results
    y = np.empty((B, N, D), dtype=np.float32)
    for b in range(B):
        y[b] = outs[2 * b]["out"] + outs[2 * b + 1]["out"]
    return y
